# revision 85
# baseline (speedup 1.0000x reference)
"""Multi-head self-attention (RoPE + causal softmax) on 8 Trainium2 NeuronCores.

Sharding: head-parallel (Megatron). Core c owns heads {2c, 2c+1}:
  - Wq/Wk/Wv column-split -> each core projects its 128 features for all
    B*S = 4096 tokens in transposed layout [feat, t] (contraction on SBUF
    partitions). All matmuls run in bf16 (1 cycle/row on the PE vs 2 for
    fp32r) with fp32 PSUM accumulation.
  - RoPE via a partition-swap permutation matmul + DVE elementwise.
  - Attention per (batch, qc) with BOTH heads fused in one unit stream.
    Scores are K=64 matmuls; head0 lives on SBUF partitions 0-63 (PE tile
    T0) and head1 on 64-127 (tile T8), so alternating the two heads'
    score matmuls at MM granularity makes the 64x128-tiled PE run them
    CONCURRENTLY on separate row strips (2x score throughput). The two
    heads' exp() fire simultaneously on different engines (one on ACT,
    one on DVE via Schraudolph int16 bit-trick), which halves the
    per-unit softmax latency that previously stalled the PE and tripped
    the HAM clock gate down to 1.2 GHz for ~100us of the run.
  - Score->exp->attnV software-pipelined with a lag of one fused unit.
    Softmax denominator comes from a ones-column appended to V in the
    same PSUM accumulation group; no max-subtraction (scores are O(1)).
  - Wo row-split -> per-core partial y in fp16; host sums the 8 partials.
    Wo row tiles drain into the unit stream as PE filler.
  - xt is pre-swizzled on the host to [ki, chunk, ko, t] so every chunk
    DMA is a contiguous 8 KiB per partition (the old [D, T] layout moved
    1 KiB strided pieces at ~85 GB/s and starved the PE at startup).
"""

from collections import deque

import numpy as np

B = 2
S = 2048
D = 1024
H = 16
HD = 64
T = B * S  # 4096
P = 128
N_CORES = 8
KT = D // P  # 8 k-tiles for the projections
N_CH = T // 512  # 8 projection chunks of 512 tokens
QC_W = 512  # attention q-chunk width
N_QC = S // QC_W  # 4 q-chunks per (batch, head)
# Mask offset: large enough that exp(0.125*(s+NEG)) == 0 in fp32 and the
# Schraudolph int16 bits stay in range (no saturation needed on the DVE
# path), small enough that -1600*SCH_A + SCH_B > -32768.
NEG = -1600.0

_CACHE = {}


def _build():
    import concourse.bass as bass
    import concourse.mybir as mybir
    from concourse import bacc
    from concourse.bass import ts
    from concourse.tile import TileContext

    F32 = mybir.dt.float32
    F16 = mybir.dt.float16
    BF16 = mybir.dt.bfloat16
    I16 = mybir.dt.int16
    I8 = mybir.dt.int8
    FP8 = mybir.dt.float8e4
    DR = mybir.MatmulPerfMode.DoubleRow
    EXP = mybir.ActivationFunctionType.Exp
    MULT = mybir.AluOpType.mult
    ADD = mybir.AluOpType.add
    # Schraudolph: bf16 bits of exp(s*0.125) ~= s*SCH_A + SCH_B as int16.
    # Mean-centered (C=-7.4); |s|<=~170 keeps the bits far from saturation.
    SCH_A = 0.125 * 1.4426950408889634 * 128.0
    SCH_B = 127.0 * 128.0 - 7.4
    # fp8e4m3 variant for the DoubleRow attnV path: bits = s*A8 + B8 as
    # int8. Valid for |s| <= ~38 (scores here are ~N(0,1)); masked tiles
    # never take this path (their exps run on ACT, where exp underflows
    # to an exact fp8 zero).
    SCH8_A = 0.125 * 1.4426950408889634 * 8.0
    SCH8_B = 7.0 * 8.0 - 7.4 / 16.0

    nc = bacc.Bacc("TRN2", target_bir_lowering=False, debug=False,
                   num_devices=N_CORES)

    xt = nc.dram_tensor("xt", [P, N_CH, KT, 512], BF16, kind="ExternalInput")
    wq = nc.dram_tensor("wq", [P, KT, P], BF16, kind="ExternalInput")
    wk = nc.dram_tensor("wk", [P, KT, P], BF16, kind="ExternalInput")
    wv = nc.dram_tensor("wv", [P, KT, P], BF16, kind="ExternalInput")
    wo = nc.dram_tensor("wo", [P, D], BF16, kind="ExternalInput")
    cos = nc.dram_tensor("cos", [P, S], BF16, kind="ExternalInput")
    sin = nc.dram_tensor("sin", [P, S], BF16, kind="ExternalInput")
    perm = nc.dram_tensor("perm", [P, P], BF16, kind="ExternalInput")
    ident = nc.dram_tensor("ident", [P, P], BF16, kind="ExternalInput")
    cmask = nc.dram_tensor("cmask", [P, P], BF16, kind="ExternalInput")
    y = nc.dram_tensor("y", [T, D], F16, kind="ExternalOutput")

    with TileContext(nc) as tc:
        with (
            tc.tile_pool(name="consts", bufs=1) as consts,
            tc.tile_pool(name="xtp", bufs=2) as xtp,
            tc.tile_pool(name="work", bufs=2) as work,
            tc.tile_pool(name="expp", bufs=6) as expp,
            tc.tile_pool(name="outp", bufs=4) as outp,
        ):
            # ---- resident tiles ----
            wq_sb = consts.tile([P, KT, P], BF16, tag="wq")
            wk_sb = consts.tile([P, KT, P], BF16, tag="wk")
            wv_sb = consts.tile([P, KT, P], BF16, tag="wv")
            wo_sb = consts.tile([P, D], BF16, tag="wo")
            cos_sb = consts.tile([P, S], BF16, tag="cos")
            sin_sb = consts.tile([P, S], BF16, tag="sin")
            perm_sb = consts.tile([P, P], BF16, tag="perm")
            id_sb = consts.tile([P, P], BF16, tag="ident")
            cm_sb = consts.tile([P, P], BF16, tag="cmask")
            rotq = consts.tile([P, T], BF16, tag="rotq")
            rotk = consts.tile([P, T], BF16, tag="rotk")
            # V in natural [kp, d] layout: [kp_part, kp_tile, head, 64 + 1 one]
            # NOTE: an fp8e4+DoubleRow attnV variant was tried (pairs two
            # k-tiles per matmul, ~10us PE): the ACT exp->fp8 path is
            # clean, but the DVE int8-Schraudolph poisons ~1e-6 of
            # elements (raw scores are N(0,8); s<-38.5 gives negative
            # int8 bits that alias to -448/NaN in fp8e4) -- rejected for
            # tail-risk of intermittent error spikes.
            vall = consts.tile([P, T // P, 2, HD + 1], BF16, tag="vall")
            ones_row = consts.tile([1, HD], BF16, tag="ones_row")
            aot = consts.tile([P, T], BF16, tag="aot")  # attn out (transposed)

            # Startup critical path: wv feeds the very first matmul (pv),
            # then the first xt chunk, then wq/wk. cos/sin arrive before
            # chunk-0's rope; wo/cmask are stage-2-only and go out during
            # chunk 1.
            xt0 = xtp.tile([P, KT, 512], BF16, tag="xt")
            nc.sync.dma_start(wv_sb[:], wv[:, :, :])
            nc.sync.dma_start(xt0[:, 0:2, :], xt[:, 0, 0:2, :])
            nc.sync.dma_start(xt0[:, 2:4, :], xt[:, 0, 2:4, :])
            nc.sync.dma_start(xt0[:, KT // 2:KT, :], xt[:, 0, KT // 2:KT, :])
            nc.sync.dma_start(wq_sb[:], wq[:, :, :])
            nc.sync.dma_start(wk_sb[:], wk[:, :, :])
            nc.sync.dma_start(perm_sb[:], perm[:, :])
            nc.sync.dma_start(id_sb[:], ident[:, :])
            nc.sync.dma_start(cos_sb[:], cos[:, :])
            nc.sync.dma_start(sin_sb[:], sin[:, :])
            nc.gpsimd.memset(ones_row[:], 1.0)
            nc.gpsimd.memset(vall[:, :, :, HD], 1.0)

            # ---- stage 1: projections + rope + V transpose ----
            stage1 = tc.tile_pool(name="pproj", bufs=1, space="PSUM")
            pproj = stage1.__enter__()
            stage1b = tc.tile_pool(name="pswp", bufs=2, space="PSUM")
            pswp = stage1b.__enter__()
            stage1c = tc.tile_pool(name="ptrp", bufs=2, space="PSUM")
            ptrp = stage1c.__enter__()
            for ch in range(N_CH):
                if ch == 0:
                    xt_t = xt0
                else:
                    xt_t = xtp.tile([P, KT, 512], BF16, tag="xt")
                    nc.sync.dma_start(xt_t[:, 0:KT // 2, :],
                                      xt[:, ch, 0:KT // 2, :])
                    nc.sync.dma_start(xt_t[:, KT // 2:KT, :],
                                      xt[:, ch, KT // 2:KT, :])
                if ch == 1:
                    # stage-2 constants, off the startup critical path
                    nc.sync.dma_start(wo_sb[:], wo[:, :])
                    nc.sync.dma_start(cm_sb[:], cmask[:, :])

                # pv first: its consumer chain (ACT copy -> PE transpose)
                # overlaps the pq/pk matmuls.
                pv = pproj.tile([P, 512], F32, tag="pv")
                pq = pproj.tile([P, 512], F32, tag="pq")
                pk = pproj.tile([P, 512], F32, tag="pk")
                for k in range(KT):
                    st, sp = (k == 0), (k == KT - 1)
                    nc.tensor.matmul(pv[:], wv_sb[:, k, :], xt_t[:, k, :],
                                     start=st, stop=sp)
                vc_t = work.tile([P, 512], BF16, tag="vchunk")
                nc.scalar.copy(vc_t[:], pv[:])
                for k in range(KT):
                    st, sp = (k == 0), (k == KT - 1)
                    nc.tensor.matmul(pq[:], wq_sb[:, k, :], xt_t[:, k, :],
                                     start=st, stop=sp)
                qc_t = work.tile([P, 512], BF16, tag="qchunk")
                nc.scalar.copy(qc_t[:], pq[:])
                for k in range(KT):
                    st, sp = (k == 0), (k == KT - 1)
                    nc.tensor.matmul(pk[:], wk_sb[:, k, :], xt_t[:, k, :],
                                     start=st, stop=sp)
                kc_t = work.tile([P, 512], BF16, tag="kchunk")
                nc.scalar.copy(kc_t[:], pk[:])

                # V: transpose [feat, t] -> natural [t, feat] via PE
                for sub in range(4):
                    ptr_t = ptrp.tile([P, P], BF16, tag="ptr")
                    nc.tensor.transpose(ptr_t[:], vc_t[:, ts(sub, P)],
                                        id_sb[:])
                    nc.scalar.copy(
                        vall[:, ch * 4 + sub, :, 0:HD],
                        ptr_t[:].rearrange("p (h d) -> p h d", h=2))

                # RoPE: rot = proj * cos + swap(proj) * sin_signed
                s_sl = ts(ch % (S // 512), 512)
                for src_ps, src_sb, dst in ((pq, qc_t, rotq),
                                            (pk, kc_t, rotk)):
                    psw = pswp.tile([P, 512], F32, tag="psw")
                    nc.tensor.matmul(psw[:], perm_sb[:], src_sb[:],
                                     start=True, stop=True)
                    t1 = work.tile([P, 512], F32, tag="ropet1")
                    t2 = work.tile([P, 512], F32, tag="ropet2")
                    nc.vector.tensor_mul(t1[:], src_ps[:], cos_sb[:, s_sl])
                    nc.vector.tensor_mul(t2[:], psw[:], sin_sb[:, s_sl])
                    # NOTE: keep this add on the DVE. Offloading it to
                    # GPSIMD measured ~3-5us faster but produced
                    # INTERMITTENT corruption (~1 in 4 runs) -- the
                    # framework's gpsimd->PE synchronization is not
                    # trustworthy for data-path ops.
                    nc.vector.tensor_add(dst[:, ts(ch, 512)], t1[:], t2[:])

            stage1c.__exit__(None, None, None)
            stage1b.__exit__(None, None, None)
            stage1.__exit__(None, None, None)

            # ---- stage 2: attention, both heads fused per unit ----
            # PSUM: pss2 ring [P,2,512] bufs=3 (6 banks; holds the per-head
            # score pairs, the denominator broadcasts and the Wo py tiles)
            # + ps_o accumulators bufs=2 (2 banks; h0/h1 of the live qc).
            stage2 = tc.tile_pool(name="pss2", bufs=3, space="PSUM")
            pssp = stage2.__enter__()
            stage2b = tc.tile_pool(name="pso", bufs=2, space="PSUM")
            psop = stage2b.__enter__()

            ycopy_flip = [0]
            fexp_ctr = [0]
            wo_queue = deque()  # (b, mi) token tiles ready for Wo
            norm_queue = deque()  # (due_uidx, closure)
            norm_done = {}  # (b, qc) -> count of heads normalized

            def emit_warm_burst(n):
                # Dense tiny matmuls to keep the HAM clock gate at 2.4GHz
                # across the pool-transition idle. Results are never read.
                warm = pssp.tile([P, 2, 512], F32, tag="pss")
                for i in range(n):
                    nc.tensor.matmul(warm[:, i % 2, 0:64], perm_sb[:],
                                     rotq[:, 0:64], start=True, stop=True,
                                     skip_group_check=True)

            def emit_wo_one():
                b, mi = wo_queue.popleft()
                m = b * (S // P) + mi
                py2 = pssp.tile([P, 2, 512], F32, tag="pss")
                for oc in range(2):
                    nc.tensor.matmul(py2[:, oc, :], aot[:, ts(m, P)],
                                     wo_sb[:, ts(oc, 512)],
                                     start=True, stop=True)
                y_sb = outp.tile([P, 2, 512], F16, tag="ysb")
                ycopy_flip[0] ^= 1
                if ycopy_flip[0]:
                    nc.scalar.copy(y_sb[:], py2[:])
                else:
                    nc.vector.tensor_copy(y_sb[:], py2[:])
                nc.sync.dma_start(y[ts(m, P), :],
                                  y_sb[:].rearrange("p a b -> p (a b)"))

            # One software-pipelined stream over all (b, qc) with both
            # heads interleaved at matmul granularity: h0 score MMs run on
            # PE row strip T0 (SBUF parts 0-63), h1 on T8 (64-127) -- the
            # row-tiled PE executes adjacent different-strip MMs
            # concurrently. attnV lags one fused unit; normalize one more;
            # Wo row tiles drain as PE filler. qc descending: each b ends
            # with the small A/B units of qc=0, shrinking the drain tail.
            units = []  # (b, qc, kind, p2)
            for b in range(B):
                for qc in reversed(range(N_QC)):
                    for p2 in range(2 * qc):
                        units.append((b, qc, "F", p2))
                    units.append((b, qc, "A", None))
                    units.append((b, qc, "B", None))

            qc_state = {}  # (b,hl,qc) -> [ps_o, seg_idx, nseg]
            uidx = [0]

            def get_qc(b, hl, qc):
                key = (b, hl, qc)
                if key not in qc_state:
                    ps_o = psop.tile([P, QC_W], F32, tag="pso")
                    qc_state[key] = [ps_o, 0, 4 * qc + 4]
                return qc_state[key]

            def exp_act(e2, ps2):
                nc.scalar.activation(e2[:], ps2[:], EXP, scale=0.125)

            def exp_dve(ei, ps2):
                nc.vector.tensor_scalar(ei[:], ps2[:], SCH_A, SCH_B,
                                        MULT, ADD)

            def emit_unit(u):
                b, qc, kind, p2 = u
                t0 = b * S
                q0 = t0 + QC_W * qc
                nt = 1 if kind == "B" else 2
                ps2 = {hl: pssp.tile([P, 2, 512], F32, tag="pss",
                                     name=f"ps2h{hl}")
                       for hl in range(nt)}
                segs = {0: [], 1: []}
                if kind == "F":
                    # tile kk holds BOTH heads' scores for k-tile
                    # t = 2*p2+kk (h0 -> bank 0, h1 -> bank 1; the two
                    # strips still run concurrently). Each tile's exp
                    # fires after only its 2 matmuls instead of the
                    # unit's 3rd/4th, so every PSUM ring slot recycles
                    # ~2 MM-times earlier. Every 4th unit swaps which
                    # k-tile goes to ACT vs DVE to balance the engines.
                    fexp_ctr[0] += 1
                    act_kk = 0 if fexp_ctr[0] % 4 else 1
                    e2s = {}
                    for kk in range(2):
                        t = 2 * p2 + kk
                        for hl in (0, 1):
                            pr = slice(HD * hl, HD * hl + HD)
                            nc.tensor.matmul(
                                ps2[kk][:, hl, :],
                                rotk[pr, t0 + P * t: t0 + P * (t + 1)],
                                rotq[pr, q0:q0 + 512],
                                start=True, stop=True)
                        if kk == act_kk:
                            e2 = expp.tile([P, 2, 512], BF16, tag="expT",
                                           bufs=6)
                            exp_act(e2, ps2[kk])
                        else:
                            ei = expp.tile([P, 2, 512], I16, tag="expI",
                                           bufs=6)
                            e2 = ei.bitcast(BF16)
                            exp_dve(ei, ps2[kk])
                        e2s[kk] = e2
                    for hl in (0, 1):
                        segs[hl] = [(e2s[kk], hl, 0, 512, 0, 2 * p2 + kk)
                                    for kk in range(2)]
                    return segs
                if kind == "A":
                    # PSUM start=True lazily zero-marks the WHOLE bank, so
                    # each region's mask must land before the next
                    # start=True re-marks that bank: keep score->mask
                    # adjacency per td, interleaving heads inside it.
                    tds = ((0, 0, 0, 512, 0),
                           (1, 1, 0, 384, 128),
                           (3, 1, 384, 512, 384))
                    for (td, h2, c0, c1, qoff) in tds:
                        t = 4 * qc + td
                        for hl in (0, 1):
                            pr = slice(HD * hl, HD * hl + HD)
                            nc.tensor.matmul(
                                ps2[hl][:, h2, c0:c1],
                                rotk[pr, t0 + P * t: t0 + P * (t + 1)],
                                rotq[pr, q0 + qoff:q0 + 512],
                                start=True, stop=False)
                            segs[hl].append((None, h2, c0, c1, qoff, t))
                        mc = c0 if td != 1 else 0
                        for hl in (0, 1):
                            nc.tensor.matmul(
                                ps2[hl][:, h2, mc:mc + P], cm_sb[:],
                                id_sb[:], start=False, stop=True)
                    for hl in (0, 1):
                        if hl == 0:
                            e2 = expp.tile([P, 2, 512], BF16, tag="expT",
                                           bufs=6)
                            exp_act(e2, ps2[hl])
                        else:
                            ei = expp.tile([P, 2, 512], I16, tag="expI",
                                           bufs=6)
                            e2 = ei.bitcast(BF16)
                            exp_dve(ei, ps2[hl])
                        segs[hl] = [(e2,) + s[1:] for s in segs[hl]]
                        segs[hl].sort(key=lambda s: s[5])
                    return segs
                # "B": both heads share ONE ring tile (h0 -> bank0,
                # h1 -> bank1) and one DVE exp, minimizing ring churn at
                # the qc boundary.
                t = 4 * qc + 2
                for hl in (0, 1):
                    pr = slice(HD * hl, HD * hl + HD)
                    nc.tensor.matmul(
                        ps2[0][:, hl, 0:256],
                        rotk[pr, t0 + P * t: t0 + P * (t + 1)],
                        rotq[pr, q0 + 256:q0 + 512],
                        start=True, stop=False)
                    nc.tensor.matmul(
                        ps2[0][:, hl, 0:P], cm_sb[:], id_sb[:],
                        start=False, stop=True)
                ei = expp.tile([P, 2, 512], I16, tag="expI", bufs=6)
                e2 = ei.bitcast(BF16)
                nc.vector.tensor_scalar(ei[:, :, 0:256],
                                        ps2[0][:, :, 0:256],
                                        SCH_A, SCH_B, MULT, ADD)
                for hl in (0, 1):
                    segs[hl].append((e2, hl, 0, 256, 256, t))
                return segs

            def emit_att(u, segs):
                b, qc, kind, _ = u
                t0 = b * S
                nmax = max(len(segs[0]), len(segs[1]))
                for i in range(nmax):
                    for hl in (0, 1):
                        if i >= len(segs[hl]):
                            continue
                        (e2, h2, c0, c1, qoff, t) = segs[hl][i]
                        st = get_qc(b, hl, qc)
                        ps_o = st[0]
                        j = st[1]
                        st[1] += 1
                        w = c1 - c0
                        nc.tensor.matmul(
                            ps_o[0:HD + 1, qoff:qoff + w],
                            vall[:, b * (S // P) + t, hl, :],
                            e2[:, h2, c0:c1],
                            start=(j == 0), stop=(j == st[2] - 1),
                            skip_group_check=True)
                        if st[1] == st[2]:
                            _finish_qc(b, hl, qc, ps_o)

            def _finish_qc(b, hl, qc, ps_o):
                # Evacuate ps_o to SBUF right away so its PSUM bank
                # recycles within ~1 unit (the deferred normalize multiply
                # otherwise holds the bank for the whole chain and stalls
                # the next qc's attnV for ~4us). The denominator broadcast
                # uses the proven PE ones-matmul path -- NOT
                # gpsimd.partition_broadcast, which raced intermittently.
                pr = slice(HD * hl, HD * hl + HD)
                t0 = b * S
                dn = work.tile([1, QC_W], BF16, tag="denr")
                aou = work.tile([HD, QC_W], F32, tag="aou")
                if hl == 0:
                    nc.scalar.copy(dn[:], ps_o[HD:HD + 1, :])
                    nc.vector.tensor_copy(aou[:], ps_o[0:HD, :])
                else:
                    nc.vector.tensor_copy(dn[:], ps_o[HD:HD + 1, :])
                    nc.scalar.copy(aou[:], ps_o[0:HD, :])
                del qc_state[(b, hl, qc)]

                def norm(qc=qc, aou=aou, dn=dn, b=b, hl=hl,
                         t0=t0, pr=pr):
                    pbt = pssp.tile([P, 2, 512], F32, tag="pss")
                    nc.tensor.matmul(pbt[0:HD, 0, :], ones_row[:],
                                     dn[:], start=True, stop=True)
                    rb_sb = work.tile([HD, QC_W], F32, tag="rbsb")
                    nc.vector.reciprocal_approx_fast(
                        rb_sb[:], pbt[0:HD, 0, :])
                    q0 = t0 + QC_W * qc
                    nc.vector.tensor_mul(aot[pr, q0:q0 + QC_W],
                                         aou[:], rb_sb[:])
                    cnt = norm_done.get((b, qc), 0) + 1
                    norm_done[(b, qc)] = cnt
                    if cnt == 2:
                        for mi in range(4 * qc, 4 * qc + 4):
                            wo_queue.append((b, mi))

                norm_queue.append([uidx[0] + 1, norm])

            pending = deque()
            # Transition bridge: a LONG warm burst from the psop pool --
            # psop's banks are free before the stage-1 evacuation
            # completes (unlike pssp's, which the burst would otherwise
            # contend), so the PE stays busy from the last chunk matmul
            # straight through the attention spin-up and the HAM gate
            # never drops to 1.2 GHz at the transition.
            warm_o = psop.tile([P, QC_W], F32, tag="pso", name="warm_o")
            for i in range(80):
                nc.tensor.matmul(warm_o[:, 0:64], perm_sb[:],
                                 rotq[:, 0:64], start=True, stop=True,
                                 skip_group_check=True)
            for u in units:
                uidx[0] += 1
                segs = emit_unit(u)
                if uidx[0] in (2, 36):
                    # 2: right after the stage-1->2 pool-transition idle;
                    # 36: before the final drain tail.
                    emit_warm_burst(20)
                while norm_queue and norm_queue[0][0] <= uidx[0]:
                    norm_queue.popleft()[1]()
                pending.append((u, segs))
                if len(pending) > 1:
                    emit_att(*pending.popleft())
                n_wo = 2 if len(wo_queue) > 4 else (1 if wo_queue else 0)
                for _ in range(n_wo):
                    if wo_queue:
                        emit_wo_one()
            emit_warm_burst(48)  # keep the clock gate hot into the drain
            while pending:
                emit_att(*pending.popleft())
                uidx[0] += 1
                while norm_queue and norm_queue[0][0] <= uidx[0]:
                    norm_queue.popleft()[1]()
                while wo_queue:
                    emit_wo_one()
            while norm_queue:
                norm_queue.popleft()[1]()
            while wo_queue:
                emit_wo_one()

            stage2b.__exit__(None, None, None)
            stage2.__exit__(None, None, None)

    nc.compile()
    return nc


def _host_prep(x, token_positions, Wq, Wk, Wv, Wo, rope_sin, rope_cos):
    import ml_dtypes
    bf16 = ml_dtypes.bfloat16

    x = np.asarray(x, dtype=np.float32)
    Wq = np.asarray(Wq, dtype=np.float32)
    Wk = np.asarray(Wk, dtype=np.float32)
    Wv = np.asarray(Wv, dtype=np.float32)
    Wo = np.asarray(Wo, dtype=np.float32)
    pos = np.asarray(token_positions).astype(np.int64)
    sin_g = np.asarray(rope_sin, dtype=np.float32)[pos]  # [S, 32]
    cos_g = np.asarray(rope_cos, dtype=np.float32)[pos]

    # xt as [ki, ch, ko, t]: every chunk DMA reads a contiguous 8 KiB per
    # partition instead of 1 KiB strided pieces.
    xt_full = x.reshape(T, D).T.astype(bf16)  # [D, T]
    xt = np.ascontiguousarray(
        xt_full.reshape(KT, P, N_CH, 512).transpose(1, 2, 0, 3))

    j = np.arange(P) % 32
    cosE = np.ascontiguousarray(cos_g.T[j, :])  # [128, S]
    sgn = np.where((np.arange(P) % HD) < 32, -1.0, 1.0).astype(np.float32)
    sinS = np.ascontiguousarray(sgn[:, None] * sin_g.T[j, :])

    p_idx = np.arange(P)
    swap = (p_idx // HD) * HD + ((p_idx % HD) + 32) % HD
    perm = np.zeros((P, P), dtype=np.float32)
    perm[swap, p_idx] = 1.0
    ident = np.eye(P, dtype=np.float32)

    # triangle mask as matmul stationary: out[p, j] += cmask[j, p] with an
    # identity moving operand; masks iff j < p (q-local j, k-local p)
    jj = np.arange(P)[:, None]
    pp = np.arange(P)[None, :]
    cmask = np.where(jj < pp, NEG, 0.0).astype(np.float32)

    in_maps = []
    for c in range(N_CORES):
        feats = []
        for hl in range(2):
            h = 2 * c + hl
            base = h * HD
            feats.extend(base + 2 * np.arange(32))      # x1 (even d)
            feats.extend(base + 2 * np.arange(32) + 1)  # x2 (odd d)
        feats = np.array(feats)
        nat = np.arange(2 * c * HD, (2 * c + 2) * HD)
        # weights pre-swizzled to [ki, ko, f] so the DMA is contiguous
        # (2 KiB per partition line) instead of 256 B strided pieces.
        def wswz(w):
            return np.ascontiguousarray(
                w.reshape(KT, P, P).transpose(1, 0, 2)).astype(bf16)

        in_maps.append({
            "xt": xt,
            "wq": wswz(Wq[feats, :].T),
            "wk": wswz(Wk[feats, :].T),
            "wv": wswz(Wv[nat, :].T),
            "wo": np.ascontiguousarray(Wo[:, nat].T).astype(bf16),
            "cos": cosE.astype(bf16), "sin": sinS.astype(bf16),
            "perm": perm.astype(bf16), "ident": ident.astype(bf16),
            "cmask": cmask.astype(bf16),
        })
    return in_maps


def run(trace=False, **inputs):
    from concourse.bass_utils import run_bass_kernel_spmd

    if "nc" not in _CACHE:
        _CACHE["nc"] = _build()
    nc = _CACHE["nc"]
    in_maps = _host_prep(**inputs)
    res = run_bass_kernel_spmd(nc, in_maps, core_ids=list(range(N_CORES)),
                               trace=trace)
    out = np.zeros((T, D), dtype=np.float32)
    for c in range(N_CORES):
        out += res.results[c]["y"].astype(np.float32)
    return out.reshape(B, S, D), res


def kernel(**inputs) -> np.ndarray:
    out, _ = run(trace=False, **inputs)
    return out


# revision 87
# speedup vs baseline: 1.1004x; 1.1004x over previous
"""Multi-head self-attention (RoPE + causal softmax) on 8 Trainium2 NeuronCores.

Sharding: head-parallel (Megatron). Core c owns heads {2c, 2c+1}:
  - Wq/Wk/Wv column-split -> each core projects its 128 features for all
    B*S = 4096 tokens in transposed layout [feat, t] (contraction on SBUF
    partitions). All matmuls run in bf16 (1 cycle/row on the PE vs 2 for
    fp32r) with fp32 PSUM accumulation.
  - RoPE via a partition-swap permutation matmul + DVE elementwise.
  - Attention per (batch, qc) with BOTH heads fused in one unit stream.
    Scores are K=64 matmuls; head0 lives on SBUF partitions 0-63 (PE tile
    T0) and head1 on 64-127 (tile T8), so alternating the two heads'
    score matmuls at MM granularity makes the 64x128-tiled PE run them
    CONCURRENTLY on separate row strips (2x score throughput). The two
    heads' exp() fire simultaneously on different engines (one on ACT,
    one on DVE via Schraudolph int16 bit-trick), which halves the
    per-unit softmax latency that previously stalled the PE and tripped
    the HAM clock gate down to 1.2 GHz for ~100us of the run.
  - Score->exp->attnV software-pipelined with a lag of one fused unit.
    Softmax denominator comes from a ones-column appended to V in the
    same PSUM accumulation group; no max-subtraction (scores are O(1)).
  - Wo row-split -> per-core partial y in fp16; host sums the 8 partials.
    Wo row tiles drain into the unit stream as PE filler.
  - xt is pre-swizzled on the host to [ki, chunk, ko, t] so every chunk
    DMA is a contiguous 8 KiB per partition (the old [D, T] layout moved
    1 KiB strided pieces at ~85 GB/s and starved the PE at startup).
"""

from collections import deque

import numpy as np

B = 2
S = 2048
D = 1024
H = 16
HD = 64
T = B * S  # 4096
P = 128
N_CORES = 8
KT = D // P  # 8 k-tiles for the projections
N_CH = T // 512  # 8 projection chunks of 512 tokens
QC_W = 512  # attention q-chunk width
N_QC = S // QC_W  # 4 q-chunks per (batch, head)
# Mask offset: large enough that exp(0.125*(s+NEG)) == 0 in fp32 and the
# Schraudolph int16 bits stay in range (no saturation needed on the DVE
# path), small enough that -1600*SCH_A + SCH_B > -32768.
NEG = -1600.0

_CACHE = {}


def _build():
    import concourse.bass as bass
    import concourse.mybir as mybir
    from concourse import bacc
    from concourse.bass import ts
    from concourse.tile import TileContext

    F32 = mybir.dt.float32
    F16 = mybir.dt.float16
    BF16 = mybir.dt.bfloat16
    I16 = mybir.dt.int16
    I8 = mybir.dt.int8
    FP8 = mybir.dt.float8e4
    DR = mybir.MatmulPerfMode.DoubleRow
    EXP = mybir.ActivationFunctionType.Exp
    MULT = mybir.AluOpType.mult
    ADD = mybir.AluOpType.add
    # Schraudolph: bf16 bits of exp(s*0.125) ~= s*SCH_A + SCH_B as int16.
    # Mean-centered (C=-7.4); |s|<=~170 keeps the bits far from saturation.
    SCH_A = 0.125 * 1.4426950408889634 * 128.0
    SCH_B = 127.0 * 128.0 - 7.4
    # fp8e4m3 variant for the DoubleRow attnV path: bits = s*A8 + B8 as
    # int8. Valid for |s| <= ~38 (scores here are ~N(0,1)); masked tiles
    # never take this path (their exps run on ACT, where exp underflows
    # to an exact fp8 zero).
    SCH8_A = 0.125 * 1.4426950408889634 * 8.0
    SCH8_B = 7.0 * 8.0 - 7.4 / 16.0

    nc = bacc.Bacc("TRN2", target_bir_lowering=False, debug=False,
                   num_devices=N_CORES)

    xt = nc.dram_tensor("xt", [P, N_CH, KT, 512], BF16, kind="ExternalInput")
    wq = nc.dram_tensor("wq", [P, KT, P], BF16, kind="ExternalInput")
    wk = nc.dram_tensor("wk", [P, KT, P], BF16, kind="ExternalInput")
    wv = nc.dram_tensor("wv", [P, KT, P], BF16, kind="ExternalInput")
    wo = nc.dram_tensor("wo", [P, D], BF16, kind="ExternalInput")
    cos = nc.dram_tensor("cos", [P, S], BF16, kind="ExternalInput")
    sin = nc.dram_tensor("sin", [P, S], BF16, kind="ExternalInput")
    perm = nc.dram_tensor("perm", [P, P], BF16, kind="ExternalInput")
    ident = nc.dram_tensor("ident", [P, P], BF16, kind="ExternalInput")
    cmask = nc.dram_tensor("cmask", [P, P], BF16, kind="ExternalInput")
    y = nc.dram_tensor("y", [T, D], F16, kind="ExternalOutput")

    with TileContext(nc) as tc:
        with (
            tc.tile_pool(name="consts", bufs=1) as consts,
            tc.tile_pool(name="xtp", bufs=2) as xtp,
            tc.tile_pool(name="work", bufs=2) as work,
            tc.tile_pool(name="expp", bufs=6) as expp,
            tc.tile_pool(name="outp", bufs=4) as outp,
        ):
            # ---- resident tiles ----
            wq_sb = consts.tile([P, KT, P], BF16, tag="wq")
            wk_sb = consts.tile([P, KT, P], BF16, tag="wk")
            wv_sb = consts.tile([P, KT, P], BF16, tag="wv")
            wo_sb = consts.tile([P, D], BF16, tag="wo")
            cos_sb = consts.tile([P, S], BF16, tag="cos")
            sin_sb = consts.tile([P, S], BF16, tag="sin")
            perm_sb = consts.tile([P, P], BF16, tag="perm")
            id_sb = consts.tile([P, P], BF16, tag="ident")
            cm_sb = consts.tile([P, P], BF16, tag="cmask")
            rotq = consts.tile([P, T], BF16, tag="rotq")
            rotk = consts.tile([P, T], BF16, tag="rotk")
            # V in natural [kp, d] layout: [kp_part, kp_tile, head, 64 + 1 one]
            # NOTE: an fp8e4+DoubleRow attnV variant was tried (pairs two
            # k-tiles per matmul, ~10us PE): the ACT exp->fp8 path is
            # clean, but the DVE int8-Schraudolph poisons ~1e-6 of
            # elements (raw scores are N(0,8); s<-38.5 gives negative
            # int8 bits that alias to -448/NaN in fp8e4) -- rejected for
            # tail-risk of intermittent error spikes.
            vall = consts.tile([P, T // P, 2, HD + 1], BF16, tag="vall")
            ones_row = consts.tile([1, HD], BF16, tag="ones_row")
            aot = consts.tile([P, T], BF16, tag="aot")  # attn out (transposed)

            # Startup critical path: wv feeds the very first matmul (pv),
            # then the first xt chunk, then wq/wk. cos/sin arrive before
            # chunk-0's rope; wo/cmask are stage-2-only and go out during
            # chunk 1.
            xt0 = xtp.tile([P, KT, 512], BF16, tag="xt")
            nc.sync.dma_start(wv_sb[:], wv[:, :, :])
            nc.sync.dma_start(xt0[:, 0:2, :], xt[:, 0, 0:2, :])
            nc.sync.dma_start(xt0[:, 2:4, :], xt[:, 0, 2:4, :])
            nc.sync.dma_start(xt0[:, KT // 2:KT, :], xt[:, 0, KT // 2:KT, :])
            nc.sync.dma_start(wq_sb[:], wq[:, :, :])
            nc.sync.dma_start(wk_sb[:], wk[:, :, :])
            nc.sync.dma_start(perm_sb[:], perm[:, :])
            nc.sync.dma_start(id_sb[:], ident[:, :])
            nc.sync.dma_start(cos_sb[:], cos[:, :])
            nc.sync.dma_start(sin_sb[:], sin[:, :])
            nc.gpsimd.memset(ones_row[:], 1.0)
            nc.gpsimd.memset(vall[:, :, :, HD], 1.0)

            # ---- stage 1: projections + rope + V transpose ----
            stage1 = tc.tile_pool(name="pproj", bufs=1, space="PSUM")
            pproj = stage1.__enter__()
            stage1b = tc.tile_pool(name="pswp", bufs=2, space="PSUM")
            pswp = stage1b.__enter__()
            stage1c = tc.tile_pool(name="ptrp", bufs=2, space="PSUM")
            ptrp = stage1c.__enter__()
            for ch in range(N_CH):
                if ch == 0:
                    xt_t = xt0
                else:
                    xt_t = xtp.tile([P, KT, 512], BF16, tag="xt")
                    nc.sync.dma_start(xt_t[:, 0:KT // 2, :],
                                      xt[:, ch, 0:KT // 2, :])
                    nc.sync.dma_start(xt_t[:, KT // 2:KT, :],
                                      xt[:, ch, KT // 2:KT, :])
                if ch == 1:
                    # stage-2 constants, off the startup critical path
                    nc.sync.dma_start(wo_sb[:], wo[:, :])
                    nc.sync.dma_start(cm_sb[:], cmask[:, :])

                # pv first: its consumer chain (ACT copy -> PE transpose)
                # overlaps the pq/pk matmuls.
                pv = pproj.tile([P, 512], F32, tag="pv")
                pq = pproj.tile([P, 512], F32, tag="pq")
                pk = pproj.tile([P, 512], F32, tag="pk")
                for k in range(KT):
                    st, sp = (k == 0), (k == KT - 1)
                    nc.tensor.matmul(pv[:], wv_sb[:, k, :], xt_t[:, k, :],
                                     start=st, stop=sp)
                vc_t = work.tile([P, 512], BF16, tag="vchunk")
                nc.scalar.copy(vc_t[:], pv[:])
                for k in range(KT):
                    st, sp = (k == 0), (k == KT - 1)
                    nc.tensor.matmul(pq[:], wq_sb[:, k, :], xt_t[:, k, :],
                                     start=st, stop=sp)
                qc_t = work.tile([P, 512], BF16, tag="qchunk")
                nc.scalar.copy(qc_t[:], pq[:])
                for k in range(KT):
                    st, sp = (k == 0), (k == KT - 1)
                    nc.tensor.matmul(pk[:], wk_sb[:, k, :], xt_t[:, k, :],
                                     start=st, stop=sp)
                kc_t = work.tile([P, 512], BF16, tag="kchunk")
                nc.scalar.copy(kc_t[:], pk[:])

                # V: transpose [feat, t] -> natural [t, feat] via PE
                for sub in range(4):
                    ptr_t = ptrp.tile([P, P], BF16, tag="ptr")
                    nc.tensor.transpose(ptr_t[:], vc_t[:, ts(sub, P)],
                                        id_sb[:])
                    nc.scalar.copy(
                        vall[:, ch * 4 + sub, :, 0:HD],
                        ptr_t[:].rearrange("p (h d) -> p h d", h=2))

                # RoPE: rot = proj * cos + swap(proj) * sin_signed
                s_sl = ts(ch % (S // 512), 512)
                for src_ps, src_sb, dst in ((pq, qc_t, rotq),
                                            (pk, kc_t, rotk)):
                    psw = pswp.tile([P, 512], F32, tag="psw")
                    nc.tensor.matmul(psw[:], perm_sb[:], src_sb[:],
                                     start=True, stop=True)
                    t1 = work.tile([P, 512], F32, tag="ropet1")
                    t2 = work.tile([P, 512], F32, tag="ropet2")
                    nc.vector.tensor_mul(t1[:], src_ps[:], cos_sb[:, s_sl])
                    nc.vector.tensor_mul(t2[:], psw[:], sin_sb[:, s_sl])
                    # NOTE: keep this add on the DVE. Offloading it to
                    # GPSIMD measured ~3-5us faster but produced
                    # INTERMITTENT corruption (~1 in 4 runs) -- the
                    # framework's gpsimd->PE synchronization is not
                    # trustworthy for data-path ops.
                    nc.vector.tensor_add(dst[:, ts(ch, 512)], t1[:], t2[:])

            stage1c.__exit__(None, None, None)
            stage1b.__exit__(None, None, None)
            stage1.__exit__(None, None, None)

            # ---- stage 2: attention, both heads fused per unit ----
            # PSUM: pss2 ring [P,2,512] bufs=3 (6 banks; holds the per-head
            # score pairs, the denominator broadcasts and the Wo py tiles)
            # + ps_o accumulators bufs=2 (2 banks; h0/h1 of the live qc).
            stage2 = tc.tile_pool(name="pss2", bufs=3, space="PSUM")
            pssp = stage2.__enter__()
            stage2b = tc.tile_pool(name="pso", bufs=2, space="PSUM")
            psop = stage2b.__enter__()

            ycopy_flip = [0]
            fexp_ctr = [0]
            wo_queue = deque()  # (b, mi) token tiles ready for Wo
            norm_queue = deque()  # (due_uidx, closure)
            norm_done = {}  # (b, qc) -> count of heads normalized

            def emit_warm_burst(n):
                # Dense tiny matmuls to keep the HAM clock gate at 2.4GHz
                # across the pool-transition idle. Results are never read.
                warm = pssp.tile([P, 2, 512], F32, tag="pss")
                for i in range(n):
                    nc.tensor.matmul(warm[:, i % 2, 0:64], perm_sb[:],
                                     rotq[:, 0:64], start=True, stop=True,
                                     skip_group_check=True)

            def emit_wo_one():
                b, mi = wo_queue.popleft()
                m = b * (S // P) + mi
                py2 = pssp.tile([P, 2, 512], F32, tag="pss")
                for oc in range(2):
                    nc.tensor.matmul(py2[:, oc, :], aot[:, ts(m, P)],
                                     wo_sb[:, ts(oc, 512)],
                                     start=True, stop=True)
                y_sb = outp.tile([P, 2, 512], F16, tag="ysb")
                ycopy_flip[0] ^= 1
                if ycopy_flip[0]:
                    nc.scalar.copy(y_sb[:], py2[:])
                else:
                    nc.vector.tensor_copy(y_sb[:], py2[:])
                nc.sync.dma_start(y[ts(m, P), :],
                                  y_sb[:].rearrange("p a b -> p (a b)"))

            # One software-pipelined stream over all (b, qc) with both
            # heads interleaved at matmul granularity: h0 score MMs run on
            # PE row strip T0 (SBUF parts 0-63), h1 on T8 (64-127) -- the
            # row-tiled PE executes adjacent different-strip MMs
            # concurrently. attnV lags one fused unit; normalize one more;
            # Wo row tiles drain as PE filler. qc descending: each b ends
            # with the small A/B units of qc=0, shrinking the drain tail.
            units = []  # (b, qc, kind, p2)
            for b in range(B):
                for qc in reversed(range(N_QC)):
                    for p2 in range(2 * qc):
                        units.append((b, qc, "F", p2))
                    units.append((b, qc, "A", None))
                    units.append((b, qc, "B", None))

            qc_state = {}  # (b,hl,qc) -> [ps_o, seg_idx, nseg]
            uidx = [0]

            def get_qc(b, hl, qc):
                key = (b, hl, qc)
                if key not in qc_state:
                    ps_o = psop.tile([P, QC_W], F32, tag="pso")
                    qc_state[key] = [ps_o, 0, 4 * qc + 4]
                return qc_state[key]

            def exp_act(e2, ps2):
                nc.scalar.activation(e2[:], ps2[:], EXP, scale=0.125)

            def exp_dve(ei, ps2):
                nc.vector.tensor_scalar(ei[:], ps2[:], SCH_A, SCH_B,
                                        MULT, ADD)

            def emit_unit(u):
                b, qc, kind, p2 = u
                t0 = b * S
                q0 = t0 + QC_W * qc
                nt = 1 if kind == "B" else 2
                ps2 = {hl: pssp.tile([P, 2, 512], F32, tag="pss",
                                     name=f"ps2h{hl}")
                       for hl in range(nt)}
                segs = {0: [], 1: []}
                if kind == "F":
                    # per-k-tile grouping: tile kk holds BOTH heads
                    # (h0 -> bank 0, h1 -> bank 1); exp fires after 2 MMs
                    fexp_ctr[0] += 1
                    act_kk = 0 if fexp_ctr[0] % 4 else 1
                    e2s = {}
                    for kk in range(2):
                        t = 2 * p2 + kk
                        for hl in (0, 1):
                            pr = slice(HD * hl, HD * hl + HD)
                            nc.tensor.matmul(
                                ps2[kk][:, hl, :],
                                rotk[pr, t0 + P * t: t0 + P * (t + 1)],
                                rotq[pr, q0:q0 + 512],
                                start=True, stop=True)
                        if kk == act_kk:
                            e2 = expp.tile([P, 2, 512], BF16, tag="expT",
                                           bufs=6)
                            exp_act(e2, ps2[kk])
                        else:
                            ei = expp.tile([P, 2, 512], I16, tag="expI",
                                           bufs=6)
                            e2 = ei.bitcast(BF16)
                            exp_dve(ei, ps2[kk])
                        e2s[kk] = e2
                    for hl in (0, 1):
                        segs[hl] = [(e2s[kk], hl, 0, 512, 0, 2 * p2 + kk)
                                    for kk in range(2)]
                    return segs
                if kind == "A":
                    # PSUM start=True lazily zero-marks the WHOLE bank, so
                    # each region's mask must land before the next
                    # start=True re-marks that bank: keep score->mask
                    # adjacency per td, interleaving heads inside it.
                    tds = ((0, 0, 0, 512, 0),
                           (1, 1, 0, 384, 128),
                           (3, 1, 384, 512, 384))
                    for (td, h2, c0, c1, qoff) in tds:
                        t = 4 * qc + td
                        for hl in (0, 1):
                            pr = slice(HD * hl, HD * hl + HD)
                            nc.tensor.matmul(
                                ps2[hl][:, h2, c0:c1],
                                rotk[pr, t0 + P * t: t0 + P * (t + 1)],
                                rotq[pr, q0 + qoff:q0 + 512],
                                start=True, stop=False)
                            segs[hl].append((None, h2, c0, c1, qoff, t))
                        mc = c0 if td != 1 else 0
                        for hl in (0, 1):
                            nc.tensor.matmul(
                                ps2[hl][:, h2, mc:mc + P], cm_sb[:],
                                id_sb[:], start=False, stop=True)
                    for hl in (0, 1):
                        if hl == 0:
                            e2 = expp.tile([P, 2, 512], BF16, tag="expT",
                                           bufs=6)
                            exp_act(e2, ps2[hl])
                        else:
                            ei = expp.tile([P, 2, 512], I16, tag="expI",
                                           bufs=6)
                            e2 = ei.bitcast(BF16)
                            exp_dve(ei, ps2[hl])
                        segs[hl] = [(e2,) + s[1:] for s in segs[hl]]
                        segs[hl].sort(key=lambda s: s[5])
                    return segs
                # "B": both heads share ONE ring tile (h0 -> bank0,
                # h1 -> bank1) and one DVE exp, minimizing ring churn at
                # the qc boundary.
                t = 4 * qc + 2
                for hl in (0, 1):
                    pr = slice(HD * hl, HD * hl + HD)
                    nc.tensor.matmul(
                        ps2[0][:, hl, 0:256],
                        rotk[pr, t0 + P * t: t0 + P * (t + 1)],
                        rotq[pr, q0 + 256:q0 + 512],
                        start=True, stop=False)
                    nc.tensor.matmul(
                        ps2[0][:, hl, 0:P], cm_sb[:], id_sb[:],
                        start=False, stop=True)
                ei = expp.tile([P, 2, 512], I16, tag="expI", bufs=6)
                e2 = ei.bitcast(BF16)
                nc.vector.tensor_scalar(ei[:, :, 0:256],
                                        ps2[0][:, :, 0:256],
                                        SCH_A, SCH_B, MULT, ADD)
                for hl in (0, 1):
                    segs[hl].append((e2, hl, 0, 256, 256, t))
                return segs

            def emit_att(u, segs):
                b, qc, kind, _ = u
                t0 = b * S
                nmax = max(len(segs[0]), len(segs[1]))
                for i in range(nmax):
                    for hl in (0, 1):
                        if i >= len(segs[hl]):
                            continue
                        (e2, h2, c0, c1, qoff, t) = segs[hl][i]
                        st = get_qc(b, hl, qc)
                        ps_o = st[0]
                        j = st[1]
                        st[1] += 1
                        w = c1 - c0
                        nc.tensor.matmul(
                            ps_o[0:HD + 1, qoff:qoff + w],
                            vall[:, b * (S // P) + t, hl, :],
                            e2[:, h2, c0:c1],
                            start=(j == 0), stop=(j == st[2] - 1),
                            skip_group_check=True)
                        if st[1] == st[2]:
                            _finish_qc(b, hl, qc, ps_o)

            def _finish_qc(b, hl, qc, ps_o):
                # Evacuate ps_o to SBUF right away so its PSUM bank
                # recycles within ~1 unit (the deferred normalize multiply
                # otherwise holds the bank for the whole chain and stalls
                # the next qc's attnV for ~4us). The denominator broadcast
                # uses the proven PE ones-matmul path -- NOT
                # gpsimd.partition_broadcast, which raced intermittently.
                pr = slice(HD * hl, HD * hl + HD)
                t0 = b * S
                dn = work.tile([1, QC_W], BF16, tag="denr")
                aou = work.tile([HD, QC_W], F32, tag="aou")
                if hl == 0:
                    nc.scalar.copy(dn[:], ps_o[HD:HD + 1, :])
                    nc.vector.tensor_copy(aou[:], ps_o[0:HD, :])
                else:
                    nc.vector.tensor_copy(dn[:], ps_o[HD:HD + 1, :])
                    nc.scalar.copy(aou[:], ps_o[0:HD, :])
                del qc_state[(b, hl, qc)]

                def norm(qc=qc, aou=aou, dn=dn, b=b, hl=hl,
                         t0=t0, pr=pr):
                    pbt = pssp.tile([P, 2, 512], F32, tag="pss")
                    nc.tensor.matmul(pbt[0:HD, 0, :], ones_row[:],
                                     dn[:], start=True, stop=True)
                    rb_sb = work.tile([HD, QC_W], F32, tag="rbsb")
                    nc.vector.reciprocal_approx_fast(
                        rb_sb[:], pbt[0:HD, 0, :])
                    q0 = t0 + QC_W * qc
                    nc.vector.tensor_mul(aot[pr, q0:q0 + QC_W],
                                         aou[:], rb_sb[:])
                    cnt = norm_done.get((b, qc), 0) + 1
                    norm_done[(b, qc)] = cnt
                    if cnt == 2:
                        for mi in range(4 * qc, 4 * qc + 4):
                            wo_queue.append((b, mi))

                norm_queue.append([uidx[0] + 1, norm])

            pending = deque()
            # Transition bridge: a LONG warm burst from the psop pool --
            # psop's banks are free before the stage-1 evacuation
            # completes (unlike pssp's, which the burst would otherwise
            # contend), so the PE stays busy from the last chunk matmul
            # straight through the attention spin-up and the HAM gate
            # never drops to 1.2 GHz at the transition.
            warm_o = psop.tile([P, QC_W], F32, tag="pso", name="warm_o")
            for i in range(80):
                nc.tensor.matmul(warm_o[:, 0:64], perm_sb[:],
                                 rotq[:, 0:64], start=True, stop=True,
                                 skip_group_check=True)
            for u in units:
                uidx[0] += 1
                segs = emit_unit(u)
                if uidx[0] in (2, 36):
                    # 2: right after the stage-1->2 pool-transition idle;
                    # 36: before the final drain tail.
                    emit_warm_burst(20)
                while norm_queue and norm_queue[0][0] <= uidx[0]:
                    norm_queue.popleft()[1]()
                pending.append((u, segs))
                if len(pending) > 1:
                    emit_att(*pending.popleft())
                n_wo = 2 if len(wo_queue) > 4 else (1 if wo_queue else 0)
                for _ in range(n_wo):
                    if wo_queue:
                        emit_wo_one()
            emit_warm_burst(48)  # keep the clock gate hot into the drain
            while pending:
                emit_att(*pending.popleft())
                uidx[0] += 1
                while norm_queue and norm_queue[0][0] <= uidx[0]:
                    norm_queue.popleft()[1]()
                while wo_queue:
                    emit_wo_one()
            while norm_queue:
                norm_queue.popleft()[1]()
            while wo_queue:
                emit_wo_one()

            stage2b.__exit__(None, None, None)
            stage2.__exit__(None, None, None)

    nc.compile()
    return nc


def _host_prep(x, token_positions, Wq, Wk, Wv, Wo, rope_sin, rope_cos):
    import ml_dtypes
    bf16 = ml_dtypes.bfloat16

    x = np.asarray(x, dtype=np.float32)
    Wq = np.asarray(Wq, dtype=np.float32)
    Wk = np.asarray(Wk, dtype=np.float32)
    Wv = np.asarray(Wv, dtype=np.float32)
    Wo = np.asarray(Wo, dtype=np.float32)
    pos = np.asarray(token_positions).astype(np.int64)
    sin_g = np.asarray(rope_sin, dtype=np.float32)[pos]  # [S, 32]
    cos_g = np.asarray(rope_cos, dtype=np.float32)[pos]

    # xt as [ki, ch, ko, t]: every chunk DMA reads a contiguous 8 KiB per
    # partition instead of 1 KiB strided pieces.
    xt_full = x.reshape(T, D).T.astype(bf16)  # [D, T]
    xt = np.ascontiguousarray(
        xt_full.reshape(KT, P, N_CH, 512).transpose(1, 2, 0, 3))

    j = np.arange(P) % 32
    cosE = np.ascontiguousarray(cos_g.T[j, :])  # [128, S]
    sgn = np.where((np.arange(P) % HD) < 32, -1.0, 1.0).astype(np.float32)
    sinS = np.ascontiguousarray(sgn[:, None] * sin_g.T[j, :])

    p_idx = np.arange(P)
    swap = (p_idx // HD) * HD + ((p_idx % HD) + 32) % HD
    perm = np.zeros((P, P), dtype=np.float32)
    perm[swap, p_idx] = 1.0
    ident = np.eye(P, dtype=np.float32)

    # triangle mask as matmul stationary: out[p, j] += cmask[j, p] with an
    # identity moving operand; masks iff j < p (q-local j, k-local p)
    jj = np.arange(P)[:, None]
    pp = np.arange(P)[None, :]
    cmask = np.where(jj < pp, NEG, 0.0).astype(np.float32)

    in_maps = []
    for c in range(N_CORES):
        feats = []
        for hl in range(2):
            h = 2 * c + hl
            base = h * HD
            feats.extend(base + 2 * np.arange(32))      # x1 (even d)
            feats.extend(base + 2 * np.arange(32) + 1)  # x2 (odd d)
        feats = np.array(feats)
        nat = np.arange(2 * c * HD, (2 * c + 2) * HD)
        # weights pre-swizzled to [ki, ko, f] so the DMA is contiguous
        # (2 KiB per partition line) instead of 256 B strided pieces.
        def wswz(w):
            return np.ascontiguousarray(
                w.reshape(KT, P, P).transpose(1, 0, 2)).astype(bf16)

        in_maps.append({
            "xt": xt,
            "wq": wswz(Wq[feats, :].T),
            "wk": wswz(Wk[feats, :].T),
            "wv": wswz(Wv[nat, :].T),
            "wo": np.ascontiguousarray(Wo[:, nat].T).astype(bf16),
            "cos": cosE.astype(bf16), "sin": sinS.astype(bf16),
            "perm": perm.astype(bf16), "ident": ident.astype(bf16),
            "cmask": cmask.astype(bf16),
        })
    return in_maps


def run(trace=False, **inputs):
    from concourse.bass_utils import run_bass_kernel_spmd

    if "nc" not in _CACHE:
        _CACHE["nc"] = _build()
    nc = _CACHE["nc"]
    in_maps = _host_prep(**inputs)
    res = run_bass_kernel_spmd(nc, in_maps, core_ids=list(range(N_CORES)),
                               trace=trace)
    out = np.zeros((T, D), dtype=np.float32)
    for c in range(N_CORES):
        out += res.results[c]["y"].astype(np.float32)
    return out.reshape(B, S, D), res


def kernel(**inputs) -> np.ndarray:
    out, _ = run(trace=False, **inputs)
    return out
